# revision 1
# baseline (speedup 1.0000x reference)
"""Causal self-attention (B=2, T=2048, D=1024, H=16, Dh=64) on 8 TRN2 cores.

Sharding: core c -> batch b = c//4 (data parallel), head group g = c%4
(tensor parallel, 4 heads = 256 dims). Each core computes a full-shape
[T, D] partial of the output projection for its (b, g); the host sums
the 4 head-group partials per batch.

Compute dtype bf16 (host-cast inputs), fp32 PSUM accumulation:
  phase 1: qT/kT [256, T] and v [T, 256] projections from xT [1024, T]
  phase 2: per head: S.T = kT_h-tile (stationary) @ qT_h over the causal
           extent, exp on ScalarE (scale=1/8), diag-tile mask on VectorE,
           then O.T[65, T] += v~_h[tk, 65] (stationary, ones col -> row
           sums l) streamed against P.T. S.T/exp and the lagged PV are
           software-pipelined so PE never waits on ScalarE. Softmax
           normalization is pipelined per 512-col region as each
           region's l completes: reshape-DMA -> exact reciprocal ->
           gpsimd partition-broadcast -> multiply.
  phase 3: out[t, :] = y.T-tiles (stationary) @ woT, accumulated over
           the two head-pair blocks in PSUM.
"""

import numpy as np
from contextlib import ExitStack

import concourse.bass as bass
import concourse.tile as tile
from concourse import bacc, mybir
from concourse.bass_utils import run_bass_kernel_spmd

F32 = mybir.dt.float32
BF16 = mybir.dt.bfloat16
CDT = BF16

B, T, D = 2, 2048, 1024
H_TOT, DH = 16, 64
HL = 4                # local heads per core
DG = HL * DH          # 256 local head dims
NT = T // 128         # 16 t-tiles
NCH = T // 512        # 4 t-chunks
CT = D // 128         # 8 c-tiles
PIECE = 1024          # S.T piece size (2 PSUM banks)

_CACHE = {}


def build():
    nc = bacc.Bacc("TRN2", target_bir_lowering=False, debug=False, num_devices=8)
    xT_d = nc.dram_tensor("xT", [NCH, 128, CT, 512], CDT, kind="ExternalInput").ap()
    wq_d = nc.dram_tensor("wq", [128, CT, DG], CDT, kind="ExternalInput").ap()
    wk_d = nc.dram_tensor("wk", [128, CT, DG], CDT, kind="ExternalInput").ap()
    wv_d = nc.dram_tensor("wv", [128, CT, DG], CDT, kind="ExternalInput").ap()
    wo_d = nc.dram_tensor("wo", [128, 2, D], CDT, kind="ExternalInput").ap()
    mask_d = nc.dram_tensor("mask", [128, 128], CDT, kind="ExternalInput").ap()
    out_d = nc.dram_tensor("out", [T, D], F32, kind="ExternalOutput").ap()

    with tile.TileContext(nc) as tc:
        with ExitStack() as ctx:
            cons = ctx.enter_context(tc.tile_pool(name="cons", bufs=1))
            xp = ctx.enter_context(tc.tile_pool(name="xp", bufs=2))
            cp = ctx.enter_context(tc.tile_pool(name="cp", bufs=3))
            pp = ctx.enter_context(tc.tile_pool(name="pp", bufs=4))
            outp = ctx.enter_context(tc.tile_pool(name="outp", bufs=4))

            # ---- constants / weights (x chunk 0 is prefetched first in
            # phase 1; bulk weights follow on split queues) ----
            wq_sb = cons.tile([128, CT, DG], CDT)
            wk_sb = cons.tile([128, CT, DG], CDT)
            wv_sb = cons.tile([128, CT, DG], CDT)
            wo_sb = cons.tile([128, 2, D], CDT)
            mask_sb = cons.tile([128, 128], CDT)

            qsb = cons.tile([128, 2, T], CDT)
            ksb = cons.tile([128, 2, T], CDT)
            lrows = cons.tile([128, T], F32)  # l row for head h at partition 32h
            v_sb = cons.tile([128, NT, HL, DH + 1], CDT)
            nc.vector.memset(v_sb[:, :, :, DH], 1.0)
            y_sb = cons.tile([128, 2, T], CDT)

            # ---- phase 1: projections ----
            with tc.tile_pool(name="ps1", bufs=4, space="PSUM") as ps1:
                for n in range(NCH):
                    x_sb = xp.tile([128, CT, 512], CDT)
                    nc.scalar.dma_start(x_sb[:, 0:4, :], xT_d[n, :, 0:4, :])
                    nc.gpsimd.dma_start(x_sb[:, 4:CT, :], xT_d[n, :, 4:CT, :])
                    if n == 0:
                        nc.sync.dma_start(wq_sb[:], wq_d[:])
                        nc.scalar.dma_start(wk_sb[:], wk_d[:])
                        nc.gpsimd.dma_start(wv_sb[:], wv_d[:])
                        nc.sync.dma_start(wo_sb[:], wo_d[:])
                        nc.sync.dma_start(mask_sb[:], mask_d[:])
                    for w_sb, dst in ((wq_sb, qsb), (wk_sb, ksb)):
                        for j2 in range(2):
                            pq = ps1.tile([128, 512], F32, tag="pq")
                            for ct in range(CT):
                                nc.tensor.matmul(
                                    pq[:],
                                    w_sb[:, ct, 128 * j2 : 128 * (j2 + 1)],
                                    x_sb[:, ct, :],
                                    start=(ct == 0),
                                    stop=(ct == CT - 1),
                                )
                            nc.vector.tensor_copy(
                                dst[:, j2, 512 * n : 512 * (n + 1)], pq[:]
                            )
                    for i in range(4):
                        ti = 4 * n + i
                        pv = ps1.tile([128, DG], F32, tag="pv")
                        for ct in range(CT):
                            nc.tensor.matmul(
                                pv[:],
                                x_sb[:, ct, 128 * i : 128 * (i + 1)],
                                wv_sb[:, ct, :],
                                start=(ct == 0),
                                stop=(ct == CT - 1),
                            )
                        nc.vector.tensor_copy(
                            v_sb[:, ti, :, 0:DH],
                            pv[:].rearrange("p (h d) -> p h d", h=HL),
                        )

            # ---- phase 2: attention, head pairs row-packed in the PE ----
            # Pair p = heads (2p, 2p+1) at partition offsets 0 / 64: their
            # K=64 S.T matmuls use disjoint row groups and run concurrently.
            # Output regions of 512 tq-cols keep O.T tiles at 1 PSUM bank per
            # head so two heads + double buffering + sT fit in 8 banks.
            with (
                tc.tile_pool(name="spool", bufs=2, space="PSUM") as spool,
                tc.tile_pool(name="opool", bufs=2, space="PSUM") as opool,
            ):
                for p in range(2):
                    for reg in range(NCH):
                        c0r, c1r = 512 * reg, 512 * (reg + 1)
                        jlast = min(NT - 1, 4 * reg + 3)
                        oTa = opool.tile(
                            [DH + 1, 512], F32, tag="oTa", name=f"oTa_{p}_{reg}"
                        )
                        oTb = opool.tile(
                            [DH + 1, 512], F32, tag="oTb", name=f"oTb_{p}_{reg}"
                        )

                        def emit_st(j):
                            c0 = max(c0r, 128 * j)
                            w = c1r - c0
                            sT = spool.tile(
                                [128, 1024], F32, tag="sT", name=f"sT{p}_{reg}_{j}"
                            )
                            nc.tensor.matmul(
                                sT[:, 0:w],
                                ksb[0:DH, p, 128 * j : 128 * (j + 1)],
                                qsb[0:DH, p, c0:c1r],
                                start=True,
                                stop=True,
                            )
                            nc.tensor.matmul(
                                sT[:, 512 : 512 + w],
                                ksb[DH:128, p, 128 * j : 128 * (j + 1)],
                                qsb[DH:128, p, c0:c1r],
                                start=True,
                                stop=True,
                            )
                            pT = pp.tile(
                                [128, 1024], CDT, tag="pT", name=f"pT{p}_{reg}_{j}"
                            )
                            nc.scalar.activation(
                                pT[:, 0 : 512 + w],
                                sT[:, 0 : 512 + w],
                                mybir.ActivationFunctionType.Exp,
                                scale=0.125,
                            )
                            if j >= 4 * reg:  # diagonal block at rel cols [0,128)
                                nc.vector.tensor_mul(
                                    pT[:, 0:128], pT[:, 0:128], mask_sb[:]
                                )
                                nc.vector.tensor_mul(
                                    pT[:, 512:640], pT[:, 512:640], mask_sb[:]
                                )
                            return pT

                        def emit_pv(j, pT):
                            c0 = max(c0r, 128 * j)
                            w = c1r - c0
                            nc.tensor.matmul(
                                oTa[:, c0 - c0r :],
                                v_sb[:, j, 2 * p, :],
                                pT[:, 0:w],
                                start=(j == 0),
                                stop=(j == jlast),
                                skip_group_check=True,
                            )
                            nc.tensor.matmul(
                                oTb[:, c0 - c0r :],
                                v_sb[:, j, 2 * p + 1, :],
                                pT[:, 512 : 512 + w],
                                start=(j == 0),
                                stop=(j == jlast),
                                skip_group_check=True,
                            )

                        def emit_norm():
                            # l final for both heads of this region: drain +
                            # normalize. lt[32reg + pp16, i] = l[c0r + 16pp + i]
                            for h, oT in ((2 * p, oTa), (2 * p + 1, oTb)):
                                hp = 64 * (h % 2)
                                nc.vector.tensor_copy(
                                    y_sb[hp : hp + DH, p, c0r:c1r], oT[0:DH, :]
                                )
                                nc.vector.tensor_copy(
                                    lrows[32 * h : 32 * h + 1, c0r:c1r],
                                    oT[DH : DH + 1, :],
                                )
                                nc.sync.dma_start(
                                    lt_sb[32 * reg : 32 * (reg + 1), :],
                                    lrows[32 * h : 32 * h + 1, c0r:c1r],
                                )
                                nc.vector.reciprocal(
                                    rt_sb[32 * reg : 32 * (reg + 1), :],
                                    lt_sb[32 * reg : 32 * (reg + 1), :],
                                )
                                nc.sync.dma_start(
                                    r_sb[:, c0r:c1r],
                                    rt_sb[32 * reg : 32 * (reg + 1), :],
                                )
                                nc.gpsimd.partition_broadcast(
                                    rb_sb[:, c0r:c1r], r_sb[:, c0r:c1r]
                                )
                                nc.vector.tensor_mul(
                                    y_sb[hp : hp + DH, p, c0r:c1r],
                                    y_sb[hp : hp + DH, p, c0r:c1r],
                                    rb_sb[hp : hp + DH, c0r:c1r],
                                )

                        lt_sb = cp.tile(
                            [128, T // 128], F32, tag="lt", name=f"lt{p}_{reg}"
                        )
                        rt_sb = cp.tile(
                            [128, T // 128], F32, tag="rt", name=f"rt{p}_{reg}"
                        )
                        r_sb = cp.tile([1, T], F32, tag="r", name=f"r{p}_{reg}")
                        rb_sb = cp.tile([128, T], F32, tag="rb", name=f"rb{p}_{reg}")

                        prev = None
                        for j in range(jlast + 1):
                            pT = emit_st(j)
                            if prev is not None:
                                emit_pv(*prev)
                            prev = (j, pT)
                        emit_pv(*prev)
                        emit_norm()

            # ---- phase 3: output projection ----
            with tc.tile_pool(name="ps3", bufs=4, space="PSUM") as ps3:
                for i in range(NT):
                    for oc in range(2):
                        po = ps3.tile([128, 512], F32, tag="po")
                        for g2 in range(2):
                            nc.tensor.matmul(
                                po[:],
                                y_sb[:, g2, 128 * i : 128 * (i + 1)],
                                wo_sb[:, g2, 512 * oc : 512 * (oc + 1)],
                                start=(g2 == 0),
                                stop=(g2 == 1),
                            )
                        o_sb = outp.tile([128, 512], F32, tag="o")
                        if (i + oc) % 2 == 0:
                            nc.vector.tensor_copy(o_sb[:], po[:])
                        else:
                            nc.scalar.copy(o_sb[:], po[:])
                        eng = nc.gpsimd if (i + oc) % 2 == 0 else nc.sync
                        eng.dma_start(
                            out_d[
                                128 * i : 128 * (i + 1), 512 * oc : 512 * (oc + 1)
                            ],
                            o_sb[:],
                        )
    nc.compile()
    return nc


def make_in_maps(x, Wq, Wk, Wv, Wo):
    import ml_dtypes

    cnp = ml_dtypes.bfloat16
    mask = np.triu(np.ones((128, 128), dtype=cnp))  # [tk, tq] valid tk<=tq
    in_maps = []
    for c in range(8):
        b, g = c // 4, c % 4
        rows = slice(DG * g, DG * (g + 1))
        in_maps.append(
            {
                "xT": np.ascontiguousarray(
                    x[b].T.reshape(CT, 128, NCH, 512).transpose(2, 1, 0, 3)
                ).astype(cnp),
                "wq": np.ascontiguousarray(
                    Wq[rows].T.reshape(CT, 128, DG).transpose(1, 0, 2)
                ).astype(cnp),
                "wk": np.ascontiguousarray(
                    Wk[rows].T.reshape(CT, 128, DG).transpose(1, 0, 2)
                ).astype(cnp),
                "wv": np.ascontiguousarray(
                    Wv[rows].T.reshape(CT, 128, DG).transpose(1, 0, 2)
                ).astype(cnp),
                "wo": np.ascontiguousarray(
                    Wo[:, rows].T.reshape(2, 128, D).transpose(1, 0, 2)
                ).astype(cnp),
                "mask": mask,
            }
        )
    return in_maps


def _run(x, Wq, Wk, Wv, Wo, trace=False):
    if "nc" not in _CACHE:
        _CACHE["nc"] = build()
    nc = _CACHE["nc"]
    in_maps = make_in_maps(x, Wq, Wk, Wv, Wo)
    res = run_bass_kernel_spmd(nc, in_maps, core_ids=list(range(8)), trace=trace)
    out = np.zeros((B, T, D), dtype=np.float32)
    for c in range(8):
        out[c // 4] += res.results[c]["out"]
    return out, res


def kernel(x, Wq, Wk, Wv, Wo):
    out, _ = _run(
        np.asarray(x, dtype=np.float32),
        np.asarray(Wq, dtype=np.float32),
        np.asarray(Wk, dtype=np.float32),
        np.asarray(Wv, dtype=np.float32),
        np.asarray(Wo, dtype=np.float32),
    )
    return out



# revision 2
# speedup vs baseline: 1.0309x; 1.0309x over previous
"""Causal self-attention (B=2, T=2048, D=1024, H=16, Dh=64) on 8 TRN2 cores.

Sharding: core c -> batch b = c//4 (data parallel), head group g = c%4
(tensor parallel, 4 heads = 256 dims). Each core computes a full-shape
[T, D] partial of the output projection for its (b, g); the host sums
the 4 head-group partials per batch.

Fused single-stream schedule (all bf16 compute, fp32 PSUM):
  The PE runs one continuous instruction stream. Attention (S.T matmul ->
  exp on ScalarE -> PV accumulate) is the primary stream; projection
  chunks 1-3 and the out-projection tiles are injected as paced "filler"
  between attention steps so the PE never idles while ScalarE chews
  through exp, and the HAM clock-gate stays warm (2.4 GHz).
  Pair 0 (heads 0,1) walks regions 0->3 while chunks arrive; pair 1
  walks 3->0 so the exp-heavy region-3 work lands while proj filler
  still exists and each region's out-projection unlocks as filler for
  the descending walk. PSUM: 2 banks shared proj/out-proj work pool +
  4 banks S.T double-buffer + 2 banks O.T accumulators = 8.
"""

import numpy as np
from collections import deque
from contextlib import ExitStack

import concourse.bass as bass
import concourse.tile as tile
from concourse import bacc, mybir
from concourse.bass_utils import run_bass_kernel_spmd

F32 = mybir.dt.float32
BF16 = mybir.dt.bfloat16
CDT = BF16

B, T, D = 2, 2048, 1024
H_TOT, DH = 16, 64
HL = 4                # local heads per core
DG = HL * DH          # 256 local head dims
NT = T // 128         # 16 t-tiles
NCH = T // 512        # 4 t-chunks / attention regions
CT = D // 128         # 8 c-tiles

_CACHE = {}


def build():
    nc = bacc.Bacc("TRN2", target_bir_lowering=False, debug=False, num_devices=8)
    xT_d = nc.dram_tensor("xT", [NCH, 128, CT, 512], CDT, kind="ExternalInput").ap()
    wq_d = nc.dram_tensor("wq", [128, CT, DG], CDT, kind="ExternalInput").ap()
    wk_d = nc.dram_tensor("wk", [128, CT, DG], CDT, kind="ExternalInput").ap()
    wv_d = nc.dram_tensor("wv", [128, CT, DG], CDT, kind="ExternalInput").ap()
    wo_d = nc.dram_tensor("wo", [128, 2, D], CDT, kind="ExternalInput").ap()
    mask_d = nc.dram_tensor("mask", [128, 128], CDT, kind="ExternalInput").ap()
    out_d = nc.dram_tensor("out", [T, D], F32, kind="ExternalOutput").ap()

    with tile.TileContext(nc) as tc:
        with ExitStack() as ctx:
            cons = ctx.enter_context(tc.tile_pool(name="cons", bufs=1))
            xp = ctx.enter_context(tc.tile_pool(name="xp", bufs=2))
            pp = ctx.enter_context(tc.tile_pool(name="pp", bufs=4))
            stg = ctx.enter_context(tc.tile_pool(name="stg", bufs=2))
            outp = ctx.enter_context(tc.tile_pool(name="outp", bufs=4))
            psw = ctx.enter_context(tc.tile_pool(name="psw", bufs=2, space="PSUM"))
            pss = ctx.enter_context(tc.tile_pool(name="pss", bufs=2, space="PSUM"))
            pso = ctx.enter_context(tc.tile_pool(name="pso", bufs=1, space="PSUM"))

            wq_sb = cons.tile([128, CT, DG], CDT)
            wk_sb = cons.tile([128, CT, DG], CDT)
            wv_sb = cons.tile([128, CT, DG], CDT)
            wo_sb = cons.tile([128, 2, D], CDT)
            mask_sb = cons.tile([128, 128], CDT)

            qsb = cons.tile([128, 2, T], CDT)
            ksb = cons.tile([128, 2, T], CDT)
            v_sb = cons.tile([128, NT, HL, DH + 1], CDT)
            nc.vector.memset(v_sb[:, :, :, DH], 1.0)
            y_sb = cons.tile([128, 2, T], CDT)

            x_tiles = {}

            def dma_x(n, split=False):
                xs = xp.tile([128, CT, 512], CDT, tag="x", name=f"x{n}")
                if split:
                    nc.scalar.dma_start(xs[:, 0:2, :], xT_d[n, :, 0:2, :])
                    nc.scalar.dma_start(xs[:, 2:4, :], xT_d[n, :, 2:4, :])
                    nc.gpsimd.dma_start(xs[:, 4:6, :], xT_d[n, :, 4:6, :])
                    nc.gpsimd.dma_start(xs[:, 6:CT, :], xT_d[n, :, 6:CT, :])
                else:
                    nc.scalar.dma_start(xs[:, 0:4, :], xT_d[n, :, 0:4, :])
                    nc.gpsimd.dma_start(xs[:, 4:CT, :], xT_d[n, :, 4:CT, :])
                x_tiles[n] = xs

            # prologue: x chunk 0 + wq first (fine-grained for PE ramp),
            # then bulk weights
            dma_x(0, split=True)
            nc.sync.dma_start(wq_sb[:, 0:4, :], wq_d[:, 0:4, :])
            nc.sync.dma_start(wq_sb[:, 4:CT, :], wq_d[:, 4:CT, :])
            nc.scalar.dma_start(wk_sb[:], wk_d[:])
            nc.gpsimd.dma_start(wv_sb[:], wv_d[:])
            nc.sync.dma_start(wo_sb[:], wo_d[:])
            nc.sync.dma_start(mask_sb[:], mask_d[:])

            # ---- filler units: (pe_cycles, emit_fn, kind) ----
            fq = deque()

            def mk_qk(n, which, j2):
                w_sb, dst = (wq_sb, qsb) if which == 0 else (wk_sb, ksb)

                def fn():
                    pq = psw.tile([128, 512], F32, tag="w", name=f"qk{n}_{which}_{j2}")
                    for ct in range(CT):
                        nc.tensor.matmul(
                            pq[:],
                            w_sb[:, ct, 128 * j2 : 128 * (j2 + 1)],
                            x_tiles[n][:, ct, :],
                            start=(ct == 0),
                            stop=(ct == CT - 1),
                            skip_group_check=True,
                        )
                    if which == 0:
                        nc.scalar.copy(dst[:, j2, 512 * n : 512 * (n + 1)], pq[:])
                    else:
                        nc.vector.tensor_copy(
                            dst[:, j2, 512 * n : 512 * (n + 1)], pq[:]
                        )

                return (4096, fn, "chunk")

            def mk_v(n, ihalf):
                def fn():
                    pv = psw.tile([128, 512], F32, tag="w", name=f"v{n}_{ihalf}")
                    for i2 in range(2):
                        i = 2 * ihalf + i2
                        for ct in range(CT):
                            nc.tensor.matmul(
                                pv[:, 256 * i2 : 256 * (i2 + 1)],
                                x_tiles[n][:, ct, 128 * i : 128 * (i + 1)],
                                wv_sb[:, ct, :],
                                start=(ct == 0),
                                stop=(ct == CT - 1),
                                skip_group_check=True,
                            )
                    for i2 in range(2):
                        ti = 4 * n + 2 * ihalf + i2
                        nc.vector.tensor_copy(
                            v_sb[:, ti, :, 0:DH],
                            pv[:, 256 * i2 : 256 * (i2 + 1)].rearrange(
                                "p (h d) -> p h d", h=HL
                            ),
                        )

                return (4096, fn, "chunk")

            def chunk_units(n):
                return (
                    [mk_qk(n, 0, j2) for j2 in range(2)]
                    + [mk_qk(n, 1, j2) for j2 in range(2)]
                    + [mk_v(n, ih) for ih in range(2)]
                )

            def mk_p3(r, i, oc):
                def fn():
                    po = psw.tile([128, 512], F32, tag="w", name=f"po{i}_{oc}")
                    for g2 in range(2):
                        nc.tensor.matmul(
                            po[:],
                            y_sb[:, g2, 128 * i : 128 * (i + 1)],
                            wo_sb[:, g2, 512 * oc : 512 * (oc + 1)],
                            start=(g2 == 0),
                            stop=(g2 == 1),
                            skip_group_check=True,
                        )
                    o_sb = outp.tile([128, 512], F32, tag="o")
                    if (i + oc) % 2 == 0:
                        nc.vector.tensor_copy(o_sb[:], po[:])
                    else:
                        nc.scalar.copy(o_sb[:], po[:])
                    eng = nc.gpsimd if (i + oc) % 2 == 0 else nc.sync
                    eng.dma_start(
                        out_d[128 * i : 128 * (i + 1), 512 * oc : 512 * (oc + 1)],
                        o_sb[:],
                    )

                return (1024, fn, "p3")

            def p3_units(r):
                return [mk_p3(r, i, oc) for i in range(4 * r, 4 * r + 4) for oc in range(2)]

            def drain(kinds=("chunk", "p3")):
                while fq and fq[0][2] in kinds:
                    _, fn, _ = fq.popleft()
                    fn()

            # ---- attention region for one head pair ----
            def region(p, reg):
                c0r, c1r = 512 * reg, 512 * (reg + 1)
                jlast = 4 * reg + 3
                ws = [c1r - max(c0r, 128 * j) for j in range(jlast + 1)]
                attn_cycles = 4 * sum(ws)
                fil_cycles = sum(c for c, _, _ in fq)
                R = fil_cycles / attn_cycles

                oTa = pso.tile([DH + 1, 512], F32, tag="oTa", name=f"oTa{p}_{reg}")
                oTb = pso.tile([DH + 1, 512], F32, tag="oTb", name=f"oTb{p}_{reg}")

                def emit_st(j):
                    c0 = max(c0r, 128 * j)
                    w = c1r - c0
                    sT = pss.tile([128, 1024], F32, tag="sT", name=f"sT{p}_{reg}_{j}")
                    nc.tensor.matmul(
                        sT[:, 0:w],
                        ksb[0:DH, p, 128 * j : 128 * (j + 1)],
                        qsb[0:DH, p, c0:c1r],
                        start=True,
                        stop=True,
                    )
                    nc.tensor.matmul(
                        sT[:, 512 : 512 + w],
                        ksb[DH:128, p, 128 * j : 128 * (j + 1)],
                        qsb[DH:128, p, c0:c1r],
                        start=True,
                        stop=True,
                    )
                    pT = pp.tile([128, 1024], CDT, tag="pT", name=f"pT{p}_{reg}_{j}")
                    nc.scalar.activation(
                        pT[:, 0 : 512 + w],
                        sT[:, 0 : 512 + w],
                        mybir.ActivationFunctionType.Exp,
                        scale=0.125,
                    )
                    if j >= 4 * reg:  # diagonal block at rel cols [0,128)
                        nc.vector.tensor_mul(pT[:, 0:128], pT[:, 0:128], mask_sb[:])
                        nc.vector.tensor_mul(
                            pT[:, 512:640], pT[:, 512:640], mask_sb[:]
                        )
                    return pT

                def emit_pv(j, pT):
                    c0 = max(c0r, 128 * j)
                    w = c1r - c0
                    nc.tensor.matmul(
                        oTa[:, c0 - c0r :],
                        v_sb[:, j, 2 * p, :],
                        pT[:, 0:w],
                        start=(j == 0),
                        stop=(j == jlast),
                        skip_group_check=True,
                    )
                    nc.tensor.matmul(
                        oTb[:, c0 - c0r :],
                        v_sb[:, j, 2 * p + 1, :],
                        pT[:, 512 : 512 + w],
                        start=(j == 0),
                        stop=(j == jlast),
                        skip_group_check=True,
                    )

                carry = 0.0
                prev = None
                for j in range(jlast + 1):
                    pT = emit_st(j)
                    carry += R * 4 * ws[j]
                    while carry > 0 and fq:
                        c, fn, _ = fq.popleft()
                        fn()
                        carry -= c
                    if prev is not None:
                        emit_pv(*prev)
                    prev = (j, pT)
                emit_pv(*prev)

                # normalization: y = O / l per head
                for hh, oT in ((0, oTa), (1, oTb)):
                    hp = 64 * hh
                    nc.vector.tensor_copy(
                        y_sb[hp : hp + DH, p, c0r:c1r], oT[0:DH, :]
                    )
                    lr = stg.tile([1, 512], F32, tag="lr", name=f"lr{p}_{reg}_{hh}")
                    nc.scalar.copy(lr[:], oT[DH : DH + 1, :])
                    lt = stg.tile([32, 16], F32, tag="lt", name=f"lt{p}_{reg}_{hh}")
                    nc.sync.dma_start(lt[:], lr[:])
                    rt = stg.tile([32, 16], F32, tag="rt", name=f"rt{p}_{reg}_{hh}")
                    nc.vector.reciprocal(rt[:], lt[:])
                    r_ = stg.tile([1, 512], F32, tag="r", name=f"r{p}_{reg}_{hh}")
                    nc.sync.dma_start(r_[:], rt[:])
                    rb = stg.tile([128, 512], F32, tag="rb", name=f"rb{p}_{reg}_{hh}")
                    nc.gpsimd.partition_broadcast(rb[:], r_[:])
                    nc.vector.tensor_mul(
                        y_sb[hp : hp + DH, p, c0r:c1r],
                        y_sb[hp : hp + DH, p, c0r:c1r],
                        rb[hp : hp + DH, :],
                    )

            # ---- main schedule ----
            for u in chunk_units(0):
                u[1]()
            dma_x(1)
            fq.extend(chunk_units(1))
            region(0, 0)
            drain(("chunk",))
            dma_x(2)
            fq.extend(chunk_units(2))
            region(0, 1)
            drain(("chunk",))
            dma_x(3)
            fq.extend(chunk_units(3))
            region(0, 2)
            drain(("chunk",))
            region(0, 3)
            region(1, 3)
            fq.extend(p3_units(3))
            region(1, 2)
            fq.extend(p3_units(2))
            region(1, 1)
            fq.extend(p3_units(1))
            region(1, 0)
            fq.extend(p3_units(0))
            drain()
    nc.compile()
    return nc


def make_in_maps(x, Wq, Wk, Wv, Wo):
    import ml_dtypes

    cnp = ml_dtypes.bfloat16
    mask = np.triu(np.ones((128, 128), dtype=cnp))  # [tk, tq] valid tk<=tq
    in_maps = []
    for c in range(8):
        b, g = c // 4, c % 4
        rows = slice(DG * g, DG * (g + 1))
        in_maps.append(
            {
                "xT": np.ascontiguousarray(
                    x[b].T.reshape(CT, 128, NCH, 512).transpose(2, 1, 0, 3)
                ).astype(cnp),
                "wq": np.ascontiguousarray(
                    Wq[rows].T.reshape(CT, 128, DG).transpose(1, 0, 2)
                ).astype(cnp),
                "wk": np.ascontiguousarray(
                    Wk[rows].T.reshape(CT, 128, DG).transpose(1, 0, 2)
                ).astype(cnp),
                "wv": np.ascontiguousarray(
                    Wv[rows].T.reshape(CT, 128, DG).transpose(1, 0, 2)
                ).astype(cnp),
                "wo": np.ascontiguousarray(
                    Wo[:, rows].T.reshape(2, 128, D).transpose(1, 0, 2)
                ).astype(cnp),
                "mask": mask,
            }
        )
    return in_maps


def _run(x, Wq, Wk, Wv, Wo, trace=False):
    if "nc" not in _CACHE:
        _CACHE["nc"] = build()
    nc = _CACHE["nc"]
    in_maps = make_in_maps(x, Wq, Wk, Wv, Wo)
    res = run_bass_kernel_spmd(nc, in_maps, core_ids=list(range(8)), trace=trace)
    out = np.zeros((B, T, D), dtype=np.float32)
    for c in range(8):
        out[c // 4] += res.results[c]["out"]
    return out, res


def kernel(x, Wq, Wk, Wv, Wo):
    out, _ = _run(
        np.asarray(x, dtype=np.float32),
        np.asarray(Wq, dtype=np.float32),
        np.asarray(Wk, dtype=np.float32),
        np.asarray(Wv, dtype=np.float32),
        np.asarray(Wo, dtype=np.float32),
    )
    return out


# revision 10
# speedup vs baseline: 1.0694x; 1.0373x over previous
"""Causal self-attention (B=2, T=2048, D=1024, H=16, Dh=64) on 8 TRN2 cores.

Sharding: core c -> batch b = c//4 (data parallel), head group g = c%4
(tensor parallel, 4 heads = 256 dims). Each core computes a full-shape
[T, D] partial of the output projection for its (b, g); the host sums
the 4 head-group partials per batch.

Fused single-stream schedule (all bf16 compute, fp32 PSUM):
  The PE runs one continuous instruction stream. Attention (S.T matmul ->
  exp on ScalarE -> PV accumulate) is the primary stream; projection
  chunks 1-3 and the out-projection tiles are injected as paced "filler"
  between attention steps so the PE never idles while ScalarE chews
  through exp, and the HAM clock-gate stays warm (2.4 GHz).
  Pair 0 (heads 0,1) walks regions 0->3 while chunks arrive; pair 1
  walks 3->0 so the exp-heavy region-3 work lands while proj filler
  still exists and each region's out-projection unlocks as filler for
  the descending walk. PSUM: 2 banks shared proj/out-proj work pool +
  4 banks S.T double-buffer + 2 banks O.T accumulators = 8.
"""

import numpy as np
from collections import deque
from contextlib import ExitStack

import concourse.bass as bass
import concourse.tile as tile
from concourse import bacc, mybir
from concourse.bass_utils import run_bass_kernel_spmd

F32 = mybir.dt.float32
BF16 = mybir.dt.bfloat16
CDT = BF16

B, T, D = 2, 2048, 1024
H_TOT, DH = 16, 64
HL = 4                # local heads per core
DG = HL * DH          # 256 local head dims
NT = T // 128         # 16 t-tiles
NCH = T // 512        # 4 t-chunks / attention regions
CT = D // 128         # 8 c-tiles

_CACHE = {}


def build():
    nc = bacc.Bacc("TRN2", target_bir_lowering=False, debug=False, num_devices=8)
    xT_d = nc.dram_tensor("xT", [NCH, 128, CT, 512], CDT, kind="ExternalInput").ap()
    wq_d = nc.dram_tensor("wq", [128, CT, DG], CDT, kind="ExternalInput").ap()
    wk_d = nc.dram_tensor("wk", [128, CT, DG], CDT, kind="ExternalInput").ap()
    wv_d = nc.dram_tensor("wv", [128, CT, DG], CDT, kind="ExternalInput").ap()
    wo_d = nc.dram_tensor("wo", [128, 2, D], CDT, kind="ExternalInput").ap()
    mask_d = nc.dram_tensor("mask", [128, 128], CDT, kind="ExternalInput").ap()
    out_d = nc.dram_tensor("out", [T, D], F32, kind="ExternalOutput").ap()

    with tile.TileContext(nc) as tc:
        with ExitStack() as ctx:
            cons = ctx.enter_context(tc.tile_pool(name="cons", bufs=1))
            xp = ctx.enter_context(tc.tile_pool(name="xp", bufs=2))
            pp = ctx.enter_context(tc.tile_pool(name="pp", bufs=4))
            stg = ctx.enter_context(tc.tile_pool(name="stg", bufs=2))
            outp = ctx.enter_context(tc.tile_pool(name="outp", bufs=4))
            psw = ctx.enter_context(tc.tile_pool(name="psw", bufs=2, space="PSUM"))
            pss = ctx.enter_context(tc.tile_pool(name="pss", bufs=2, space="PSUM"))
            pso = ctx.enter_context(tc.tile_pool(name="pso", bufs=1, space="PSUM"))

            wq_sb = cons.tile([128, CT, DG], CDT)
            wk_sb = cons.tile([128, CT, DG], CDT)
            wv_sb = cons.tile([128, CT, DG], CDT)
            wo_sb = cons.tile([128, 2, D], CDT)
            mask_sb = cons.tile([128, 128], CDT)

            qsb = cons.tile([128, 2, T], CDT)
            ksb = cons.tile([128, 2, T], CDT)
            v_sb = cons.tile([128, NT, HL, DH + 1], CDT)
            nc.vector.memset(v_sb[:, :, :, DH], 1.0)
            y_sb = cons.tile([128, 2, T], CDT)

            x_tiles = {}

            def dma_x(n, split=False):
                xs = xp.tile([128, CT, 512], CDT, tag="x", name=f"x{n}")
                if split:
                    nc.scalar.dma_start(xs[:, 0:2, :], xT_d[n, :, 0:2, :])
                    nc.scalar.dma_start(xs[:, 2:4, :], xT_d[n, :, 2:4, :])
                    nc.gpsimd.dma_start(xs[:, 4:6, :], xT_d[n, :, 4:6, :])
                    nc.gpsimd.dma_start(xs[:, 6:CT, :], xT_d[n, :, 6:CT, :])
                else:
                    nc.scalar.dma_start(xs[:, 0:4, :], xT_d[n, :, 0:4, :])
                    nc.gpsimd.dma_start(xs[:, 4:CT, :], xT_d[n, :, 4:CT, :])
                x_tiles[n] = xs

            # prologue: x chunk 0 + wq first (fine-grained for PE ramp),
            # then bulk weights
            dma_x(0, split=True)
            nc.sync.dma_start(wq_sb[:, 0:2, :], wq_d[:, 0:2, :])
            nc.sync.dma_start(wq_sb[:, 2:4, :], wq_d[:, 2:4, :])
            nc.sync.dma_start(wq_sb[:, 4:CT, :], wq_d[:, 4:CT, :])
            nc.scalar.dma_start(wk_sb[:], wk_d[:])
            nc.gpsimd.dma_start(wv_sb[:], wv_d[:])
            nc.sync.dma_start(wo_sb[:], wo_d[:])
            nc.sync.dma_start(mask_sb[:], mask_d[:])

            # ---- filler units: (pe_cycles, emit_fn, kind) ----
            fq = deque()

            def mk_qk(n, which, j2):
                w_sb, dst = (wq_sb, qsb) if which == 0 else (wk_sb, ksb)

                def fn():
                    pq = psw.tile([128, 512], F32, tag="w", name=f"qk{n}_{which}_{j2}")
                    for ct in range(CT):
                        nc.tensor.matmul(
                            pq[:],
                            w_sb[:, ct, 128 * j2 : 128 * (j2 + 1)],
                            x_tiles[n][:, ct, :],
                            start=(ct == 0),
                            stop=(ct == CT - 1),
                            skip_group_check=True,
                        )
                    nc.vector.tensor_copy(
                        dst[:, j2, 512 * n : 512 * (n + 1)], pq[:]
                    )

                return (4096, fn, "chunk")

            def mk_v(n, ihalf):
                def fn():
                    pv = psw.tile([128, 512], F32, tag="w", name=f"v{n}_{ihalf}")
                    for i2 in range(2):
                        i = 2 * ihalf + i2
                        for ct in range(CT):
                            nc.tensor.matmul(
                                pv[:, 256 * i2 : 256 * (i2 + 1)],
                                x_tiles[n][:, ct, 128 * i : 128 * (i + 1)],
                                wv_sb[:, ct, :],
                                start=(ct == 0),
                                stop=(ct == CT - 1),
                                skip_group_check=True,
                            )
                    for i2 in range(2):
                        ti = 4 * n + 2 * ihalf + i2
                        nc.vector.tensor_copy(
                            v_sb[:, ti, :, 0:DH],
                            pv[:, 256 * i2 : 256 * (i2 + 1)].rearrange(
                                "p (h d) -> p h d", h=HL
                            ),
                        )

                return (4096, fn, "chunk")

            def chunk_units(n):
                return (
                    [mk_qk(n, 0, j2) for j2 in range(2)]
                    + [mk_qk(n, 1, j2) for j2 in range(2)]
                    + [mk_v(n, ih) for ih in range(2)]
                )

            def mk_p3(r, i, oc):
                def fn():
                    po = psw.tile([128, 512], F32, tag="w", name=f"po{i}_{oc}")
                    for g2 in range(2):
                        nc.tensor.matmul(
                            po[:],
                            y_sb[:, g2, 128 * i : 128 * (i + 1)],
                            wo_sb[:, g2, 512 * oc : 512 * (oc + 1)],
                            start=(g2 == 0),
                            stop=(g2 == 1),
                            skip_group_check=True,
                        )
                    o_sb = outp.tile([128, 512], F32, tag="o")
                    nc.vector.tensor_copy(o_sb[:], po[:])
                    eng = nc.gpsimd if (i + oc) % 2 == 0 else nc.sync
                    eng.dma_start(
                        out_d[128 * i : 128 * (i + 1), 512 * oc : 512 * (oc + 1)],
                        o_sb[:],
                    )

                return (1024, fn, "p3")

            def p3_units(r):
                return [mk_p3(r, i, oc) for i in range(4 * r, 4 * r + 4) for oc in range(2)]

            def drain(kinds=("chunk", "p3")):
                while fq and fq[0][2] in kinds:
                    _, fn, _ = fq.popleft()
                    fn()

            # ---- attention region for one head pair ----
            def region(p, reg, fil_budget=None):
                c0r, c1r = 512 * reg, 512 * (reg + 1)
                jlast = 4 * reg + 3
                ws = [c1r - max(c0r, 128 * j) for j in range(jlast + 1)]
                attn_cycles = 4 * sum(ws)
                fil_cycles = sum(c for c, _, _ in fq)
                if fil_budget is not None:
                    fil_cycles = min(fil_cycles, fil_budget)
                R = fil_cycles / attn_cycles

                oTa = pso.tile([DH + 1, 512], F32, tag="oTa", name=f"oTa{p}_{reg}")
                oTb = pso.tile([DH + 1, 512], F32, tag="oTb", name=f"oTb{p}_{reg}")

                def emit_st(j):
                    c0 = max(c0r, 128 * j)
                    w = c1r - c0
                    sT = pss.tile([128, 1024], F32, tag="sT", name=f"sT{p}_{reg}_{j}")
                    nc.tensor.matmul(
                        sT[:, 0:w],
                        ksb[0:DH, p, 128 * j : 128 * (j + 1)],
                        qsb[0:DH, p, c0:c1r],
                        start=True,
                        stop=True,
                    )
                    nc.tensor.matmul(
                        sT[:, 512 : 512 + w],
                        ksb[DH:128, p, 128 * j : 128 * (j + 1)],
                        qsb[DH:128, p, c0:c1r],
                        start=True,
                        stop=True,
                    )
                    pT = pp.tile([128, 1024], CDT, tag="pT", name=f"pT{p}_{reg}_{j}")
                    nc.scalar.activation(
                        pT[:, 0 : 512 + w],
                        sT[:, 0 : 512 + w],
                        mybir.ActivationFunctionType.Exp,
                        scale=0.125,
                    )
                    if j >= 4 * reg:  # diagonal block at rel cols [0,128)
                        nc.vector.tensor_mul(pT[:, 0:128], pT[:, 0:128], mask_sb[:])
                        nc.vector.tensor_mul(
                            pT[:, 512:640], pT[:, 512:640], mask_sb[:]
                        )
                    return pT

                def emit_pv(j, pT):
                    c0 = max(c0r, 128 * j)
                    w = c1r - c0
                    nc.tensor.matmul(
                        oTa[:, c0 - c0r :],
                        v_sb[:, j, 2 * p, :],
                        pT[:, 0:w],
                        start=(j == 0),
                        stop=(j == jlast),
                        skip_group_check=True,
                    )
                    nc.tensor.matmul(
                        oTb[:, c0 - c0r :],
                        v_sb[:, j, 2 * p + 1, :],
                        pT[:, 512 : 512 + w],
                        start=(j == 0),
                        stop=(j == jlast),
                        skip_group_check=True,
                    )

                carry = 0.0
                prev = None
                for j in range(jlast + 1):
                    pT = emit_st(j)
                    carry += R * 4 * ws[j]
                    while carry > 0 and fq:
                        c, fn, _ = fq.popleft()
                        fn()
                        carry -= c
                    if prev is not None:
                        emit_pv(*prev)
                    prev = (j, pT)
                emit_pv(*prev)

                # normalization: y = O / l per head
                for hh, oT in ((0, oTa), (1, oTb)):
                    hp = 64 * hh
                    nc.vector.tensor_copy(
                        y_sb[hp : hp + DH, p, c0r:c1r], oT[0:DH, :]
                    )
                    lr = stg.tile([1, 512], F32, tag="lr", name=f"lr{p}_{reg}_{hh}")
                    nc.vector.tensor_copy(lr[:], oT[DH : DH + 1, :])
                    lt = stg.tile([32, 16], F32, tag="lt", name=f"lt{p}_{reg}_{hh}")
                    nc.sync.dma_start(lt[:], lr[:])
                    rt = stg.tile([32, 16], F32, tag="rt", name=f"rt{p}_{reg}_{hh}")
                    nc.vector.reciprocal(rt[:], lt[:])
                    r_ = stg.tile([1, 512], F32, tag="r", name=f"r{p}_{reg}_{hh}")
                    nc.sync.dma_start(r_[:], rt[:])
                    rb = stg.tile([128, 512], F32, tag="rb", name=f"rb{p}_{reg}_{hh}")
                    nc.gpsimd.partition_broadcast(rb[:], r_[:])
                    nc.vector.tensor_mul(
                        y_sb[hp : hp + DH, p, c0r:c1r],
                        y_sb[hp : hp + DH, p, c0r:c1r],
                        rb[hp : hp + DH, :],
                    )

            # ---- main schedule ----
            # pair 0 ascends regions with next-chunk projections as filler
            # (chunk 3 deliberately spills into region (0,3), the one region
            # whose exp exceeds its own attention PE work); pair 1 ascends
            # with region r-1's out-projection as filler.
            for u in chunk_units(0):
                u[1]()
            dma_x(1)
            fq.extend(chunk_units(1))
            region(0, 0)
            drain(("chunk",))
            dma_x(2)
            fq.extend(chunk_units(2))
            region(0, 1)
            drain(("chunk",))
            dma_x(3)
            fq.extend(chunk_units(3))
            region(0, 2, fil_budget=16384)
            region(0, 3)
            drain(("chunk",))
            region(1, 0)
            fq.extend(p3_units(0))
            region(1, 1)
            fq.extend(p3_units(1))
            region(1, 2)
            fq.extend(p3_units(2))
            region(1, 3)
            fq.extend(p3_units(3))
            drain()
    nc.compile()
    return nc


def make_in_maps(x, Wq, Wk, Wv, Wo):
    import ml_dtypes

    cnp = ml_dtypes.bfloat16
    mask = np.triu(np.ones((128, 128), dtype=cnp))  # [tk, tq] valid tk<=tq
    in_maps = []
    for c in range(8):
        b, g = c // 4, c % 4
        rows = slice(DG * g, DG * (g + 1))
        in_maps.append(
            {
                "xT": np.ascontiguousarray(
                    x[b].T.reshape(CT, 128, NCH, 512).transpose(2, 1, 0, 3)
                ).astype(cnp),
                "wq": np.ascontiguousarray(
                    Wq[rows].T.reshape(CT, 128, DG).transpose(1, 0, 2)
                ).astype(cnp),
                "wk": np.ascontiguousarray(
                    Wk[rows].T.reshape(CT, 128, DG).transpose(1, 0, 2)
                ).astype(cnp),
                "wv": np.ascontiguousarray(
                    Wv[rows].T.reshape(CT, 128, DG).transpose(1, 0, 2)
                ).astype(cnp),
                "wo": np.ascontiguousarray(
                    Wo[:, rows].T.reshape(2, 128, D).transpose(1, 0, 2)
                ).astype(cnp),
                "mask": mask,
            }
        )
    return in_maps


def _run(x, Wq, Wk, Wv, Wo, trace=False):
    if "nc" not in _CACHE:
        _CACHE["nc"] = build()
    nc = _CACHE["nc"]
    in_maps = make_in_maps(x, Wq, Wk, Wv, Wo)
    res = run_bass_kernel_spmd(nc, in_maps, core_ids=list(range(8)), trace=trace)
    out = np.zeros((B, T, D), dtype=np.float32)
    for c in range(8):
        out[c // 4] += res.results[c]["out"]
    return out, res


def kernel(x, Wq, Wk, Wv, Wo):
    out, _ = _run(
        np.asarray(x, dtype=np.float32),
        np.asarray(Wq, dtype=np.float32),
        np.asarray(Wk, dtype=np.float32),
        np.asarray(Wv, dtype=np.float32),
        np.asarray(Wo, dtype=np.float32),
    )
    return out
